# revision 21
# baseline (speedup 1.0000x reference)
"""Bahdanau attention kernel for 8 Trainium2 NeuronCores.

Problem (hardcoded shapes): B=32, T=8192, D_ENC=256, D_HID=512, D_ATT=512.
    proj = encoder_out @ w1 + b1 + (h @ w2 + b2) + (c @ w3 + b3)   # [B,T,512]
    scores = tanh(proj) @ wv (+ bv)                                # [B,T,1]
    attn = softmax(scores, axis=T)
    context = sum_t attn * encoder_out                             # [B,256]

Sharding: data-parallel over batch, 4 batches per core, no collectives.

Device strategy (per core, per batch):
  - encoder_out is fed twice: transposed [256,8192] in fp8e4 for the
    projection matmul, natural [8192,257] in bf16 (appended ones column)
    for the context accumulation.
  - Pass A (8 chunks of 1024 timesteps): hidden^T[j] = (16*w1)[.,j]^T @
    encT via ONE DoubleRow fp8 matmul per (j, half) (K=256 packed as
    [128,2]); tanh with scale=1/16 and the per-batch bias fused in the
    ACT instruction, output in fp8e4 into per-chunk j-pair tiles
    [128, 2, 1024].
  - Scores on PE in fp8 DoubleRow: stationary selector tiles
    [128, 2(k-pair), 2(h-col)] hold 16*wv split as wv_hi + wv_lo
    (fp8 value + fp8 residual, summed in PSUM f32, so wv carries no
    fp8 quantization error); 8 matmuls per chunk accumulate a [2, 512]
    PSUM tile (row h = scores of half h).  A small DVE cast stages the
    [2,512] rows to bf16, a scatter-DMA (DRAM bounce) drops them into
    a per-batch [128, 64] column tile, and one ACT exp per half batch
    (scale=1/16) produces e in bf16.  Scores are O(1) so no max
    subtraction is needed; bv cancels in softmax and is dropped.
  - Pass B: per 128 timesteps, acc += e_col * encN via the fused
    scalar_tensor_tensor op; even 1024-groups run on the Vector engine,
    odd groups on the (otherwise idle) GpSimd/Pool engine, each with its
    own accumulator.  The ones column of encN makes the same op
    accumulate Z = sum(e).  Finally ctx/Z via a ones^T @ acc matmul,
    reciprocal, and scale.  The very last half-batch accumulates on the
    (otherwise idle) PE instead, shrinking the end-of-kernel tail.
  Pass B lags pass A by half a batch, so the end-of-kernel exposed tail
  is only half a batch of accumulation work.
"""

import os
import sys

for _p in ("/opt/trn_rl_repo", "/root/.axon_site", "/root/.axon_site/_ro/pypackages"):
    if os.path.isdir(_p) and _p not in sys.path:
        sys.path.append(_p)

import numpy as np
import ml_dtypes

import concourse.bass as bass
import concourse.tile as tile
from concourse import bacc, bass_isa, mybir
from concourse.bass_utils import run_bass_kernel_spmd

BF16 = ml_dtypes.bfloat16
FP8 = ml_dtypes.float8_e4m3

B, T, D_ENC, D_HID, D_ATT = 32, 8192, 256, 512, 512
N_CORES = 8
BPC = B // N_CORES          # batches per core = 4
P = 128                     # partitions
TC = 1024                   # pass-A chunk (timesteps)
HTC = TC // 2               # matmul moving-dim half = 512
NCH = T // TC               # pass-A chunks per batch = 8
HCH = NCH // 2              # chunks per half-batch = 4
NU = TC // P                # 128-blocks per chunk = 8
NCOL = T // P               # score columns per batch = 64
KD = D_ENC // P             # k-subtiles of the contraction dim = 2
NJ = D_ATT // P             # a-tiles = 4
DE1 = D_ENC + 1             # encN row with ones column = 257
WSCALE = 16.0               # host-side w1/wv scale (fp8 subnormal dodge)

_PROGRAM_CACHE = {}


def _build_program():
    """Build and finalize the SPMD program (identical on all 8 cores)."""
    if "nc" in _PROGRAM_CACHE:
        return _PROGRAM_CACHE["nc"]

    f32 = mybir.dt.float32
    bf16 = mybir.dt.bfloat16
    fp8 = mybir.dt.float8e4
    Act = mybir.ActivationFunctionType
    Alu = mybir.AluOpType
    DR = mybir.MatmulPerfMode.DoubleRow

    nc = bacc.Bacc("TRN2", target_bir_lowering=False, debug=False,
                   num_devices=N_CORES)

    encT = nc.dram_tensor("encT", [BPC, D_ENC, T], bf16, kind="ExternalInput")
    encT8 = nc.dram_tensor("encT8", [BPC, D_ENC, T], fp8, kind="ExternalInput")
    encN = nc.dram_tensor("encN", [BPC, T, DE1], bf16, kind="ExternalInput")
    w1t = nc.dram_tensor("w1t", [P, KD, NJ, P], bf16, kind="ExternalInput")
    w18t = nc.dram_tensor("w18t", [P, KD, P], fp8, kind="ExternalInput")
    # selector stationaries: [p, jp, h, s(k-pair), m(48: hi at col h,
    # lo at col 32+h)] -- 48 pad keeps the DR pair stride 16B-aligned.
    wvt = nc.dram_tensor("wvt", [P, 2, 2, KD, 48], fp8, kind="ExternalInput")
    vbt = nc.dram_tensor("vbt", [P, BPC * NJ], f32, kind="ExternalInput")
    outd = nc.dram_tensor("out", [BPC, D_ENC], f32, kind="ExternalOutput")
    sscr = nc.dram_tensor("sscr", [BPC * NCH, 2, HTC], bf16)

    with tile.TileContext(nc) as tc:
        import contextlib
        with contextlib.ExitStack() as ctx:
            const = ctx.enter_context(tc.tile_pool(name="const", bufs=1))
            encT_pool = ctx.enter_context(tc.tile_pool(name="encT", bufs=6))
            encT8_pool = ctx.enter_context(tc.tile_pool(name="encT8", bufs=6))
            encN_pool = ctx.enter_context(tc.tile_pool(name="encN", bufs=4))
            tanh_pool = ctx.enter_context(tc.tile_pool(name="tanh", bufs=6))
            ssb_pool = ctx.enter_context(tc.tile_pool(name="ssb", bufs=4))
            stg_pool = ctx.enter_context(tc.tile_pool(name="stg", bufs=2))
            e_pool = ctx.enter_context(tc.tile_pool(name="e", bufs=2))
            sm_pool = ctx.enter_context(tc.tile_pool(name="sm", bufs=4))
            osb_pool = ctx.enter_context(tc.tile_pool(name="osb", bufs=2))
            accv_pool = ctx.enter_context(tc.tile_pool(name="accv", bufs=2))
            accg_pool = ctx.enter_context(tc.tile_pool(name="accg", bufs=2))
            hid_psum = ctx.enter_context(
                tc.tile_pool(name="hid", bufs=3, space="PSUM"))
            sc_psum = ctx.enter_context(
                tc.tile_pool(name="sc", bufs=1, space="PSUM"))
            cf_psum = ctx.enter_context(
                tc.tile_pool(name="cfin", bufs=1, space="PSUM"))

            # constants
            w1_sb = const.tile([P, KD, NJ, P], bf16)
            nc.scalar.dma_start(w1_sb[:], w1t[:])
            w18_sb = const.tile([P, KD, P], fp8)
            nc.scalar.dma_start(w18_sb[:], w18t[:])
            wvt_sb = const.tile([P, 2, 2, KD, 48], fp8)
            nc.scalar.dma_start(wvt_sb[:], wvt[:])
            vbt_sb = const.tile([P, BPC * NJ], f32)
            nc.scalar.dma_start(vbt_sb[:], vbt[:])
            ones128 = const.tile([P, 1], f32)
            nc.gpsimd.memset(ones128[:], 1.0)

            stage = {}   # per-batch [128, 64] bf16: scattered 16*score cols
            e_sb = {}    # per-batch [128, 64] bf16: exp(scores)
            acc_v = {}   # per-batch [128, 257] f32: DVE accumulator (even g)
            acc_g = {}   # per-batch [128, 257] f32: DVE accumulator (odd g)
            cfp = {}     # last batch: PE-accumulated [1, 257] psum
            tanh_of = {}  # chunk -> (pair tile jp=0, pair tile jp=1)

            def emit_A_main(b, i):
                encT_t = encT_pool.tile([P, KD, TC], bf16)
                src_ap = (encT[b, :, i * TC:(i + 1) * TC]
                          .rearrange("(k p) t -> p k t", p=P))
                if b == 0 and i < 2:
                    # cold-start: split the first chunks across two queues
                    nc.sync.dma_start(encT_t[:, 0, :], src_ap[:, 0, :])
                    nc.gpsimd.dma_start(encT_t[:, 1, :], src_ap[:, 1, :])
                else:
                    nc.sync.dma_start(encT_t[:], src_ap)
                pair_tiles = []
                for jp in range(NJ // 2):
                    tp = tanh_pool.tile([P, 2, TC], fp8, tag="tanh")
                    pair_tiles.append(tp)
                encT8_t = encT8_pool.tile([P, KD, TC], fp8)
                nc.gpsimd.dma_start(
                    encT8_t[:],
                    encT8[b, :, i * TC:(i + 1) * TC]
                        .rearrange("(k p) t -> p k t", p=P))
                for j in range(NJ):
                    h_ps = hid_psum.tile([P, TC], f32, tag="hid")
                    if j == NJ - 1:
                        # last j-tile in fp8 DoubleRow (error budget allows
                        # one of four; halves this tile's matmul count)
                        for h in range(2):
                            nc.tensor.matmul(
                                h_ps[:, h * HTC:(h + 1) * HTC],
                                w18_sb[:],
                                encT8_t[:, :, h * HTC:(h + 1) * HTC],
                                start=True, stop=True, perf_mode=DR)
                    else:
                        for k in range(KD):
                            for h in range(2):
                                nc.tensor.matmul(
                                    h_ps[:, h * HTC:(h + 1) * HTC],
                                    w1_sb[:, k, j, :],
                                    encT_t[:, k, h * HTC:(h + 1) * HTC],
                                    start=(k == 0), stop=(k == KD - 1))
                    nc.scalar.activation(
                        pair_tiles[j // 2][:, j % 2, :], h_ps[:], Act.Tanh,
                        scale=(1.0 / WSCALE if j == NJ - 1 else 1.0),
                        bias=vbt_sb[:, b * NJ + j: b * NJ + j + 1])
                tanh_of[i] = pair_tiles

            def emit_A_scores(b, i):
                pair_tiles = tanh_of.pop(i)
                s_ps = sc_psum.tile([34, HTC], f32, tag="sc")
                first, last = (0, 0), (1, 1)
                for h in range(2):
                    for jp in range(NJ // 2):
                        nc.tensor.matmul(
                            s_ps[:],
                            wvt_sb[:, jp, h, :, 0:34],
                            pair_tiles[jp][:, :, h * HTC:(h + 1) * HTC],
                            start=((h, jp) == first),
                            stop=((h, jp) == last),
                            perf_mode=DR)
                # psum rows 0:2 = 16*s_hi (h0,h1), rows 32:34 = 16*s_lo;
                # engines may read only one PSUM operand per op, so stage
                # hi to SBUF first (ACT/DVE alternating), then add lo.
                # Scatter drops the bf16 rows into column form:
                # col m = i*8 + h*4 + u holds scores for t = m*128 + p.
                s_hi = ssb_pool.tile([2, HTC], bf16, tag="shi")
                if i % 2 == 0 or b == BPC - 1:
                    nc.scalar.activation(s_hi[:], s_ps[0:2, :], Act.Copy)
                else:
                    nc.vector.tensor_copy(s_hi[:], s_ps[0:2, :])
                s_sb = ssb_pool.tile([2, HTC], bf16, tag="ssb")
                nc.vector.tensor_add(s_sb[:], s_ps[32:34, :], s_hi[:])
                row = sscr[b * NCH + i]
                nc.sync.dma_start(row, s_sb[:])
                nc.sync.dma_start(
                    stage[b][:, i * NU:(i + 1) * NU],
                    row.rearrange("h (u p) -> p (h u)", p=P))
                if b == BPC - 1 and i >= HCH:
                    # last batch tail: per-chunk exp so the PE tail can
                    # start as soon as each chunk's columns land.
                    nc.scalar.activation(
                        e_sb[b][:, i * NU:(i + 1) * NU],
                        stage[b][:, i * NU:(i + 1) * NU],
                        Act.Exp, scale=1.0 / WSCALE)

            def emit_exp_half(b, half):
                nc.scalar.activation(
                    e_sb[b][:, half * 32:(half + 1) * 32],
                    stage[b][:, half * 32:(half + 1) * 32],
                    Act.Exp, scale=1.0 / WSCALE)

            def emit_acc_init(b):
                acc_v[b] = accv_pool.tile([P, DE1], f32, tag="accv",
                                          name=f"acc_v{b}")
                nc.vector.memset(acc_v[b][:], 0.0)
                acc_g[b] = accg_pool.tile([P, DE1], f32, tag="accg",
                                          name=f"acc_g{b}")
                nc.gpsimd.memset(acc_g[b][:], 0.0)


            def emit_B_group(b, g):
                """One pass-B group = super-chunk g (1024 timesteps)."""
                encN_t = encN_pool.tile([P, NU, DE1], bf16)
                nc.sync.dma_start(
                    encN_t[:],
                    encN[b, g * TC:(g + 1) * TC, :]
                        .rearrange("(n p) d -> p n d", p=P))
                if b == BPC - 1 and g >= 2:
                    # PE is otherwise idle in the kernel tail: accumulate
                    # these groups directly in PSUM via matmuls.
                    if g == 2:
                        cfp["t"] = cf_psum.tile([1, DE1], f32, tag="cfin",
                                                name="cfp_last")
                    for n in range(NU):
                        m = NU * g + n
                        nc.tensor.matmul(
                            cfp["t"][:],
                            e_sb[b][:, m:m + 1],
                            encN_t[:, n, :],
                            start=(g == 2 and n == 0), stop=False)
                    return
                acc = acc_v if g % 2 == 0 else acc_g
                for n in range(NU):
                    m = NU * g + n
                    nc.vector.scalar_tensor_tensor(
                        acc[b][:], encN_t[:, n, :],
                        e_sb[b][:, m:m + 1],
                        acc[b][:],
                        op0=Alu.mult, op1=Alu.add)

            def emit_B_finalize(b):
                if b == BPC - 1:
                    cf = cfp["t"]
                    nc.tensor.matmul(cf[:], ones128[:], acc_v[b][:],
                                     start=False, stop=False)
                    nc.tensor.matmul(cf[:], ones128[:], acc_g[b][:],
                                     start=False, stop=True)
                else:
                    cf = cf_psum.tile([1, DE1], f32, tag="cfin")
                    nc.tensor.matmul(cf[:], ones128[:], acc_v[b][:],
                                     start=True, stop=False)
                    nc.tensor.matmul(cf[:], ones128[:], acc_g[b][:],
                                     start=False, stop=True)
                rzb = sm_pool.tile([1, 1], f32, tag="rz", name=f"rz{b}")
                nc.vector.reciprocal(rzb[:], cf[:, D_ENC:D_ENC + 1])
                o_sb = osb_pool.tile([1, D_ENC], f32, tag="osb")
                nc.vector.tensor_scalar_mul(o_sb[:], cf[:, 0:D_ENC], rzb[:])
                nc.sync.dma_start(outd[b:b + 1, :], o_sb[:])

            for step in range(BPC + 1):
                if step < BPC:
                    stage[step] = stg_pool.tile([P, NCOL], bf16, tag="stg",
                                                name=f"stage{step}")
                    e_sb[step] = e_pool.tile([P, NCOL], bf16, tag="e",
                                             name=f"e_sb{step}")
                for i in range(NCH):
                    if step < BPC:
                        if i > 0:
                            emit_A_scores(step, i - 1)
                        emit_A_main(step, i)
                        if i == HCH:
                            emit_acc_init(step)
                            emit_exp_half(step, 0)
                    if i < HCH:
                        if step >= 1:
                            emit_B_group(step - 1, HCH + i)
                            if i == HCH - 1:
                                emit_B_finalize(step - 1)
                    else:
                        if step < BPC:
                            emit_B_group(step, i - HCH)
                if step < BPC:
                    emit_A_scores(step, NCH - 1)
                    if step != BPC - 1:
                        emit_exp_half(step, 1)

    nc.finalize()
    _PROGRAM_CACHE["nc"] = nc
    return nc


def _prep_inputs(encoder_out, hidden_state_h, hidden_state_c,
                 w1, b1, w2, b2, w3, b3, wv, bv):
    """Host-side sharding + layout prep. Returns per-core input maps."""
    enc = np.asarray(encoder_out, dtype=np.float32)
    # per-batch bias vector: b1 + h@w2 + b2 + c@w3 + b3  (tiny, exact f32)
    vb = (np.asarray(b1, np.float32)
          + np.asarray(hidden_state_h, np.float32) @ np.asarray(w2, np.float32)
          + np.asarray(b2, np.float32)
          + np.asarray(hidden_state_c, np.float32) @ np.asarray(w3, np.float32)
          + np.asarray(b3, np.float32))                        # [B, D_ATT]
    # bv shifts every score equally -> cancels in softmax; dropped.

    w1_h = np.ascontiguousarray(
        np.asarray(w1, np.float32).reshape(KD, P, NJ, P).transpose(1, 0, 2, 3)
    ).astype(BF16)                                             # [128,2,4,128]
    w18_h = np.ascontiguousarray(
        (np.asarray(w1, np.float32)[:, (NJ - 1) * P:] * WSCALE)
        .reshape(KD, P, P).transpose(1, 0, 2)
    ).astype(FP8)                                              # [128,2,128]

    # 16*wv split into fp8 value + fp8 residual; selector layout
    # [p, jp, h, s, m]: col h holds hi of wv[(2*jp+s)*128+p], col 32+h
    # holds the lo residual (summed post-PSUM by the cast's tensor_add).
    wv_f = np.asarray(wv, np.float32).reshape(-1) * WSCALE     # [512]
    wv_hi = wv_f.astype(FP8)
    wv_lo = (wv_f - wv_hi.astype(np.float32)).astype(FP8)
    wvsel = np.zeros((P, 2, 2, KD, 48), np.float32)
    for jp in range(2):
        for s in range(KD):
            a0 = (2 * jp + s) * P
            for h in range(2):
                wvsel[:, jp, h, s, h] = wv_hi[a0:a0 + P].astype(np.float32)
                wvsel[:, jp, h, s, 32 + h] = wv_lo[a0:a0 + P].astype(np.float32)
    wvsel = wvsel.astype(FP8)

    in_maps = []
    for c in range(N_CORES):
        sl = slice(c * BPC, (c + 1) * BPC)
        enc_c = enc[sl]                                        # [4, T, 256]
        encT_cf = np.ascontiguousarray(enc_c.transpose(0, 2, 1))
        encT_c = encT_cf.astype(BF16)
        encT8_c = encT_cf.astype(FP8)
        encN_c = np.ascontiguousarray(np.concatenate(
            [enc_c, np.ones((BPC, T, 1), np.float32)], axis=2)).astype(BF16)
        vbt_c = np.ascontiguousarray(
            vb[sl].reshape(BPC, NJ, P).transpose(2, 0, 1).reshape(P, BPC * NJ)
        ).astype(np.float32)
        in_maps.append({
            "encT": encT_c,
            "encT8": encT8_c,
            "w18t": w18_h,
            "encN": encN_c,
            "w1t": w1_h,
            "wvt": wvsel,
            "vbt": vbt_c,
        })
    return in_maps


def kernel(**inputs):
    nc = _build_program()
    in_maps = _prep_inputs(**inputs)
    res = run_bass_kernel_spmd(nc, in_maps, list(range(N_CORES)))
    out = np.concatenate([res.results[c]["out"] for c in range(N_CORES)],
                         axis=0)
    return out.astype(np.float32)


if __name__ == "__main__":
    rng = np.random.default_rng(0)
    ins = {
        "encoder_out": rng.standard_normal((B, T, D_ENC), dtype=np.float32),
        "hidden_state_h": rng.standard_normal((B, D_HID), dtype=np.float32),
        "hidden_state_c": rng.standard_normal((B, D_HID), dtype=np.float32),
        "w1": (rng.standard_normal((D_ENC, D_ATT), dtype=np.float32)
               / np.sqrt(D_ENC)),
        "b1": np.zeros(D_ATT, np.float32),
        "w2": (rng.standard_normal((D_HID, D_ATT), dtype=np.float32)
               / np.sqrt(D_HID)),
        "b2": np.zeros(D_ATT, np.float32),
        "w3": (rng.standard_normal((D_HID, D_ATT), dtype=np.float32)
               / np.sqrt(D_HID)),
        "wv": (rng.standard_normal((D_ATT, 1), dtype=np.float32)
               / np.sqrt(D_ATT)),
        "b3": np.zeros(D_ATT, np.float32),
        "bv": np.zeros(1, np.float32),
    }
    got = kernel(**ins)
    print("kernel output:", got.shape, got.dtype)


# revision 22
# speedup vs baseline: 1.1328x; 1.1328x over previous
"""Bahdanau attention kernel for 8 Trainium2 NeuronCores.

Problem (hardcoded shapes): B=32, T=8192, D_ENC=256, D_HID=512, D_ATT=512.
    proj = encoder_out @ w1 + b1 + (h @ w2 + b2) + (c @ w3 + b3)   # [B,T,512]
    scores = tanh(proj) @ wv (+ bv)                                # [B,T,1]
    attn = softmax(scores, axis=T)
    context = sum_t attn * encoder_out                             # [B,256]

Sharding: data-parallel over batch, 4 batches per core, no collectives.

Device strategy (per core, per batch):
  - encoder_out is fed twice: transposed [256,8192] in fp8e4 for the
    projection matmul, natural [8192,257] in bf16 (appended ones column)
    for the context accumulation.
  - Pass A (8 chunks of 1024 timesteps): hidden^T[j] = (16*w1)[.,j]^T @
    encT via ONE DoubleRow fp8 matmul per (j, half) (K=256 packed as
    [128,2]); tanh with scale=1/16 and the per-batch bias fused in the
    ACT instruction, output in fp8e4 into per-chunk j-pair tiles
    [128, 2, 1024].
  - Scores on PE in fp8 DoubleRow: stationary selector tiles
    [128, 2(k-pair), 2(h-col)] hold 16*wv split as wv_hi + wv_lo
    (fp8 value + fp8 residual, summed in PSUM f32, so wv carries no
    fp8 quantization error); 8 matmuls per chunk accumulate a [2, 512]
    PSUM tile (row h = scores of half h).  A small DVE cast stages the
    [2,512] rows to bf16, a scatter-DMA (DRAM bounce) drops them into
    a per-batch [128, 64] column tile, and one ACT exp per half batch
    (scale=1/16) produces e in bf16.  Scores are O(1) so no max
    subtraction is needed; bv cancels in softmax and is dropped.
  - Pass B: per 128 timesteps, acc += e_col * encN via the fused
    scalar_tensor_tensor op; even 1024-groups run on the Vector engine,
    odd groups on the (otherwise idle) GpSimd/Pool engine, each with its
    own accumulator.  The ones column of encN makes the same op
    accumulate Z = sum(e).  Finally ctx/Z via a ones^T @ acc matmul,
    reciprocal, and scale.  The very last half-batch accumulates on the
    (otherwise idle) PE instead, shrinking the end-of-kernel tail.
  Pass B lags pass A by half a batch, so the end-of-kernel exposed tail
  is only half a batch of accumulation work.
"""

import os
import sys

for _p in ("/opt/trn_rl_repo", "/root/.axon_site", "/root/.axon_site/_ro/pypackages"):
    if os.path.isdir(_p) and _p not in sys.path:
        sys.path.append(_p)

import numpy as np
import ml_dtypes

import concourse.bass as bass
import concourse.tile as tile
from concourse import bacc, bass_isa, mybir
from concourse.bass_utils import run_bass_kernel_spmd

BF16 = ml_dtypes.bfloat16
FP8 = ml_dtypes.float8_e4m3

B, T, D_ENC, D_HID, D_ATT = 32, 8192, 256, 512, 512
N_CORES = 8
BPC = B // N_CORES          # batches per core = 4
P = 128                     # partitions
TC = 1024                   # pass-A chunk (timesteps)
HTC = TC // 2               # matmul moving-dim half = 512
NCH = T // TC               # pass-A chunks per batch = 8
HCH = NCH // 2              # chunks per half-batch = 4
NU = TC // P                # 128-blocks per chunk = 8
NCOL = T // P               # score columns per batch = 64
KD = D_ENC // P             # k-subtiles of the contraction dim = 2
NJ = D_ATT // P             # a-tiles = 4
DE1 = D_ENC + 1             # encN row with ones column = 257
WSCALE = 16.0               # host-side w1/wv scale (fp8 subnormal dodge)

_PROGRAM_CACHE = {}


def _build_program():
    """Build and finalize the SPMD program (identical on all 8 cores)."""
    if "nc" in _PROGRAM_CACHE:
        return _PROGRAM_CACHE["nc"]

    f32 = mybir.dt.float32
    bf16 = mybir.dt.bfloat16
    fp8 = mybir.dt.float8e4
    Act = mybir.ActivationFunctionType
    Alu = mybir.AluOpType
    DR = mybir.MatmulPerfMode.DoubleRow

    nc = bacc.Bacc("TRN2", target_bir_lowering=False, debug=False,
                   num_devices=N_CORES)

    encT = nc.dram_tensor("encT", [BPC, D_ENC, T], bf16, kind="ExternalInput")
    encT8 = nc.dram_tensor("encT8", [BPC, D_ENC, T], fp8, kind="ExternalInput")
    encN = nc.dram_tensor("encN", [BPC, T, DE1], bf16, kind="ExternalInput")
    w1t = nc.dram_tensor("w1t", [P, KD, NJ, P], bf16, kind="ExternalInput")
    w18t = nc.dram_tensor("w18t", [P, KD, P], fp8, kind="ExternalInput")
    # selector stationaries: [p, jp, h, s(k-pair), m(48: hi at col h,
    # lo at col 32+h)] -- 48 pad keeps the DR pair stride 16B-aligned.
    wvt = nc.dram_tensor("wvt", [P, 2, 2, KD, 48], fp8, kind="ExternalInput")
    vbt = nc.dram_tensor("vbt", [P, BPC * NJ], f32, kind="ExternalInput")
    outd = nc.dram_tensor("out", [BPC, D_ENC], f32, kind="ExternalOutput")
    sscr = nc.dram_tensor("sscr", [BPC * NCH, 2, HTC], bf16)

    with tile.TileContext(nc) as tc:
        import contextlib
        with contextlib.ExitStack() as ctx:
            const = ctx.enter_context(tc.tile_pool(name="const", bufs=1))
            encT_pool = ctx.enter_context(tc.tile_pool(name="encT", bufs=6))
            encT8_pool = ctx.enter_context(tc.tile_pool(name="encT8", bufs=6))
            encN_pool = ctx.enter_context(tc.tile_pool(name="encN", bufs=4))
            tanh_pool = ctx.enter_context(tc.tile_pool(name="tanh", bufs=6))
            ssb_pool = ctx.enter_context(tc.tile_pool(name="ssb", bufs=4))
            stg_pool = ctx.enter_context(tc.tile_pool(name="stg", bufs=2))
            e_pool = ctx.enter_context(tc.tile_pool(name="e", bufs=2))
            sm_pool = ctx.enter_context(tc.tile_pool(name="sm", bufs=4))
            osb_pool = ctx.enter_context(tc.tile_pool(name="osb", bufs=2))
            accv_pool = ctx.enter_context(tc.tile_pool(name="accv", bufs=2))
            accg_pool = ctx.enter_context(tc.tile_pool(name="accg", bufs=2))
            hid_psum = ctx.enter_context(
                tc.tile_pool(name="hid", bufs=3, space="PSUM"))
            sc_psum = ctx.enter_context(
                tc.tile_pool(name="sc", bufs=1, space="PSUM"))
            cf_psum = ctx.enter_context(
                tc.tile_pool(name="cfin", bufs=1, space="PSUM"))

            # constants
            w1_sb = const.tile([P, KD, NJ, P], bf16)
            nc.scalar.dma_start(w1_sb[:], w1t[:])
            w18_sb = const.tile([P, KD, P], fp8)
            nc.scalar.dma_start(w18_sb[:], w18t[:])
            wvt_sb = const.tile([P, 2, 2, KD, 48], fp8)
            nc.scalar.dma_start(wvt_sb[:], wvt[:])
            vbt_sb = const.tile([P, BPC * NJ], f32)
            nc.scalar.dma_start(vbt_sb[:], vbt[:])
            ones128 = const.tile([P, 1], f32)
            nc.gpsimd.memset(ones128[:], 1.0)

            stage = {}   # per-batch [128, 64] bf16: scattered 16*score cols
            e_sb = {}    # per-batch [128, 64] bf16: exp(scores)
            acc_v = {}   # per-batch [128, 257] f32: DVE accumulator (even g)
            acc_g = {}   # per-batch [128, 257] f32: DVE accumulator (odd g)
            cfp = {}     # last batch: PE-accumulated [1, 257] psum
            tanh_of = {}  # chunk -> (pair tile jp=0, pair tile jp=1)

            def emit_A_main(b, i):
                encT_t = encT_pool.tile([P, KD, TC], bf16)
                src_ap = (encT[b, :, i * TC:(i + 1) * TC]
                          .rearrange("(k p) t -> p k t", p=P))
                if b == 0 and i < 2:
                    # cold-start: split the first chunks across two queues
                    nc.sync.dma_start(encT_t[:, 0, :], src_ap[:, 0, :])
                    nc.gpsimd.dma_start(encT_t[:, 1, :], src_ap[:, 1, :])
                else:
                    nc.sync.dma_start(encT_t[:], src_ap)
                pair_tiles = []
                for jp in range(NJ // 2):
                    tp = tanh_pool.tile([P, 2, TC], fp8, tag="tanh")
                    pair_tiles.append(tp)
                encT8_t = encT8_pool.tile([P, KD, TC], fp8)
                nc.sync.dma_start(
                    encT8_t[:],
                    encT8[b, :, i * TC:(i + 1) * TC]
                        .rearrange("(k p) t -> p k t", p=P))
                for j in range(NJ):
                    h_ps = hid_psum.tile([P, TC], f32, tag="hid")
                    if j == NJ - 1:
                        # last j-tile in fp8 DoubleRow (error budget allows
                        # one of four; halves this tile's matmul count)
                        for h in range(2):
                            nc.tensor.matmul(
                                h_ps[:, h * HTC:(h + 1) * HTC],
                                w18_sb[:],
                                encT8_t[:, :, h * HTC:(h + 1) * HTC],
                                start=True, stop=True, perf_mode=DR)
                    else:
                        for k in range(KD):
                            for h in range(2):
                                nc.tensor.matmul(
                                    h_ps[:, h * HTC:(h + 1) * HTC],
                                    w1_sb[:, k, j, :],
                                    encT_t[:, k, h * HTC:(h + 1) * HTC],
                                    start=(k == 0), stop=(k == KD - 1))
                    nc.scalar.activation(
                        pair_tiles[j // 2][:, j % 2, :], h_ps[:], Act.Tanh,
                        scale=(1.0 / WSCALE if j == NJ - 1 else 1.0),
                        bias=vbt_sb[:, b * NJ + j: b * NJ + j + 1])
                tanh_of[i] = pair_tiles

            def emit_A_scores(b, i):
                pair_tiles = tanh_of.pop(i)
                s_ps = sc_psum.tile([34, HTC], f32, tag="sc")
                first, last = (0, 0), (1, 1)
                for h in range(2):
                    for jp in range(NJ // 2):
                        nc.tensor.matmul(
                            s_ps[:],
                            wvt_sb[:, jp, h, :, 0:34],
                            pair_tiles[jp][:, :, h * HTC:(h + 1) * HTC],
                            start=((h, jp) == first),
                            stop=((h, jp) == last),
                            perf_mode=DR)
                # psum rows 0:2 = 16*s_hi (h0,h1), rows 32:34 = 16*s_lo;
                # engines may read only one PSUM operand per op, so stage
                # hi to SBUF first (ACT/DVE alternating), then add lo.
                # Scatter drops the bf16 rows into column form:
                # col m = i*8 + h*4 + u holds scores for t = m*128 + p.
                s_hi = ssb_pool.tile([2, HTC], bf16, tag="shi")
                if i % 2 == 0 or b == BPC - 1:
                    nc.scalar.activation(s_hi[:], s_ps[0:2, :], Act.Copy)
                else:
                    nc.vector.tensor_copy(s_hi[:], s_ps[0:2, :])
                s_sb = ssb_pool.tile([2, HTC], bf16, tag="ssb")
                nc.vector.tensor_add(s_sb[:], s_ps[32:34, :], s_hi[:])
                row = sscr[b * NCH + i]
                nc.sync.dma_start(row, s_sb[:])
                nc.sync.dma_start(
                    stage[b][:, i * NU:(i + 1) * NU],
                    row.rearrange("h (u p) -> p (h u)", p=P))
                if b == BPC - 1 and i >= HCH:
                    # last batch tail: per-chunk exp so the PE tail can
                    # start as soon as each chunk's columns land.
                    nc.scalar.activation(
                        e_sb[b][:, i * NU:(i + 1) * NU],
                        stage[b][:, i * NU:(i + 1) * NU],
                        Act.Exp, scale=1.0 / WSCALE)

            def emit_exp_half(b, half):
                nc.scalar.activation(
                    e_sb[b][:, half * 32:(half + 1) * 32],
                    stage[b][:, half * 32:(half + 1) * 32],
                    Act.Exp, scale=1.0 / WSCALE)

            def emit_acc_init(b):
                acc_v[b] = accv_pool.tile([P, DE1], f32, tag="accv",
                                          name=f"acc_v{b}")
                nc.vector.memset(acc_v[b][:], 0.0)
                acc_g[b] = accg_pool.tile([P, DE1], f32, tag="accg",
                                          name=f"acc_g{b}")
                nc.gpsimd.memset(acc_g[b][:], 0.0)


            def emit_B_group(b, g):
                """One pass-B group = super-chunk g (1024 timesteps)."""
                encN_t = encN_pool.tile([P, NU, DE1], bf16)
                nc.sync.dma_start(
                    encN_t[:],
                    encN[b, g * TC:(g + 1) * TC, :]
                        .rearrange("(n p) d -> p n d", p=P))
                if b == BPC - 1 and g >= 2:
                    # PE is otherwise idle in the kernel tail: accumulate
                    # these groups directly in PSUM via matmuls.
                    if g == 2:
                        cfp["t"] = cf_psum.tile([1, DE1], f32, tag="cfin",
                                                name="cfp_last")
                    for n in range(NU):
                        m = NU * g + n
                        nc.tensor.matmul(
                            cfp["t"][:],
                            e_sb[b][:, m:m + 1],
                            encN_t[:, n, :],
                            start=(g == 2 and n == 0), stop=False)
                    return
                acc = acc_v if g % 2 == 0 else acc_g
                for n in range(NU):
                    m = NU * g + n
                    nc.vector.scalar_tensor_tensor(
                        acc[b][:], encN_t[:, n, :],
                        e_sb[b][:, m:m + 1],
                        acc[b][:],
                        op0=Alu.mult, op1=Alu.add)

            def emit_B_finalize(b):
                if b == BPC - 1:
                    cf = cfp["t"]
                    nc.tensor.matmul(cf[:], ones128[:], acc_v[b][:],
                                     start=False, stop=False)
                    nc.tensor.matmul(cf[:], ones128[:], acc_g[b][:],
                                     start=False, stop=True)
                else:
                    cf = cf_psum.tile([1, DE1], f32, tag="cfin")
                    nc.tensor.matmul(cf[:], ones128[:], acc_v[b][:],
                                     start=True, stop=False)
                    nc.tensor.matmul(cf[:], ones128[:], acc_g[b][:],
                                     start=False, stop=True)
                rzb = sm_pool.tile([1, 1], f32, tag="rz", name=f"rz{b}")
                nc.vector.reciprocal(rzb[:], cf[:, D_ENC:D_ENC + 1])
                o_sb = osb_pool.tile([1, D_ENC], f32, tag="osb")
                nc.vector.tensor_scalar_mul(o_sb[:], cf[:, 0:D_ENC], rzb[:])
                nc.sync.dma_start(outd[b:b + 1, :], o_sb[:])

            for step in range(BPC + 1):
                if step < BPC:
                    stage[step] = stg_pool.tile([P, NCOL], bf16, tag="stg",
                                                name=f"stage{step}")
                    e_sb[step] = e_pool.tile([P, NCOL], bf16, tag="e",
                                             name=f"e_sb{step}")
                for i in range(NCH):
                    if step < BPC:
                        if i > 0:
                            emit_A_scores(step, i - 1)
                        emit_A_main(step, i)
                        if i == HCH:
                            emit_acc_init(step)
                            emit_exp_half(step, 0)
                    if i < HCH:
                        if step >= 1:
                            emit_B_group(step - 1, HCH + i)
                            if i == HCH - 1:
                                emit_B_finalize(step - 1)
                    else:
                        if step < BPC:
                            emit_B_group(step, i - HCH)
                if step < BPC:
                    emit_A_scores(step, NCH - 1)
                    if step != BPC - 1:
                        emit_exp_half(step, 1)

    nc.finalize()
    _PROGRAM_CACHE["nc"] = nc
    return nc


def _prep_inputs(encoder_out, hidden_state_h, hidden_state_c,
                 w1, b1, w2, b2, w3, b3, wv, bv):
    """Host-side sharding + layout prep. Returns per-core input maps."""
    enc = np.asarray(encoder_out, dtype=np.float32)
    # per-batch bias vector: b1 + h@w2 + b2 + c@w3 + b3  (tiny, exact f32)
    vb = (np.asarray(b1, np.float32)
          + np.asarray(hidden_state_h, np.float32) @ np.asarray(w2, np.float32)
          + np.asarray(b2, np.float32)
          + np.asarray(hidden_state_c, np.float32) @ np.asarray(w3, np.float32)
          + np.asarray(b3, np.float32))                        # [B, D_ATT]
    # bv shifts every score equally -> cancels in softmax; dropped.

    w1_h = np.ascontiguousarray(
        np.asarray(w1, np.float32).reshape(KD, P, NJ, P).transpose(1, 0, 2, 3)
    ).astype(BF16)                                             # [128,2,4,128]
    w18_h = np.ascontiguousarray(
        (np.asarray(w1, np.float32)[:, (NJ - 1) * P:] * WSCALE)
        .reshape(KD, P, P).transpose(1, 0, 2)
    ).astype(FP8)                                              # [128,2,128]

    # 16*wv split into fp8 value + fp8 residual; selector layout
    # [p, jp, h, s, m]: col h holds hi of wv[(2*jp+s)*128+p], col 32+h
    # holds the lo residual (summed post-PSUM by the cast's tensor_add).
    wv_f = np.asarray(wv, np.float32).reshape(-1) * WSCALE     # [512]
    wv_hi = wv_f.astype(FP8)
    wv_lo = (wv_f - wv_hi.astype(np.float32)).astype(FP8)
    wvsel = np.zeros((P, 2, 2, KD, 48), np.float32)
    for jp in range(2):
        for s in range(KD):
            a0 = (2 * jp + s) * P
            for h in range(2):
                wvsel[:, jp, h, s, h] = wv_hi[a0:a0 + P].astype(np.float32)
                wvsel[:, jp, h, s, 32 + h] = wv_lo[a0:a0 + P].astype(np.float32)
    wvsel = wvsel.astype(FP8)

    in_maps = []
    for c in range(N_CORES):
        sl = slice(c * BPC, (c + 1) * BPC)
        enc_c = enc[sl]                                        # [4, T, 256]
        encT_cf = np.ascontiguousarray(enc_c.transpose(0, 2, 1))
        encT_c = encT_cf.astype(BF16)
        encT8_c = encT_cf.astype(FP8)
        encN_c = np.ascontiguousarray(np.concatenate(
            [enc_c, np.ones((BPC, T, 1), np.float32)], axis=2)).astype(BF16)
        vbt_c = np.ascontiguousarray(
            vb[sl].reshape(BPC, NJ, P).transpose(2, 0, 1).reshape(P, BPC * NJ)
        ).astype(np.float32)
        in_maps.append({
            "encT": encT_c,
            "encT8": encT8_c,
            "w18t": w18_h,
            "encN": encN_c,
            "w1t": w1_h,
            "wvt": wvsel,
            "vbt": vbt_c,
        })
    return in_maps


def kernel(**inputs):
    nc = _build_program()
    in_maps = _prep_inputs(**inputs)
    res = run_bass_kernel_spmd(nc, in_maps, list(range(N_CORES)))
    out = np.concatenate([res.results[c]["out"] for c in range(N_CORES)],
                         axis=0)
    return out.astype(np.float32)


if __name__ == "__main__":
    rng = np.random.default_rng(0)
    ins = {
        "encoder_out": rng.standard_normal((B, T, D_ENC), dtype=np.float32),
        "hidden_state_h": rng.standard_normal((B, D_HID), dtype=np.float32),
        "hidden_state_c": rng.standard_normal((B, D_HID), dtype=np.float32),
        "w1": (rng.standard_normal((D_ENC, D_ATT), dtype=np.float32)
               / np.sqrt(D_ENC)),
        "b1": np.zeros(D_ATT, np.float32),
        "w2": (rng.standard_normal((D_HID, D_ATT), dtype=np.float32)
               / np.sqrt(D_HID)),
        "b2": np.zeros(D_ATT, np.float32),
        "w3": (rng.standard_normal((D_HID, D_ATT), dtype=np.float32)
               / np.sqrt(D_HID)),
        "wv": (rng.standard_normal((D_ATT, 1), dtype=np.float32)
               / np.sqrt(D_ATT)),
        "b3": np.zeros(D_ATT, np.float32),
        "bv": np.zeros(1, np.float32),
    }
    got = kernel(**ins)
    print("kernel output:", got.shape, got.dtype)


# revision 23
# speedup vs baseline: 1.1619x; 1.0256x over previous
"""Bahdanau attention kernel for 8 Trainium2 NeuronCores.

Problem (hardcoded shapes): B=32, T=8192, D_ENC=256, D_HID=512, D_ATT=512.
    proj = encoder_out @ w1 + b1 + (h @ w2 + b2) + (c @ w3 + b3)   # [B,T,512]
    scores = tanh(proj) @ wv (+ bv)                                # [B,T,1]
    attn = softmax(scores, axis=T)
    context = sum_t attn * encoder_out                             # [B,256]

Sharding: data-parallel over batch, 4 batches per core, no collectives.

Device strategy (per core, per batch):
  - encoder_out is fed twice: transposed [256,8192] in fp8e4 for the
    projection matmul, natural [8192,257] in bf16 (appended ones column)
    for the context accumulation.
  - Pass A (8 chunks of 1024 timesteps): hidden^T[j] = (16*w1)[.,j]^T @
    encT via ONE DoubleRow fp8 matmul per (j, half) (K=256 packed as
    [128,2]); tanh with scale=1/16 and the per-batch bias fused in the
    ACT instruction, output in fp8e4 into per-chunk j-pair tiles
    [128, 2, 1024].
  - Scores on PE in fp8 DoubleRow: stationary selector tiles
    [128, 2(k-pair), 2(h-col)] hold 16*wv split as wv_hi + wv_lo
    (fp8 value + fp8 residual, summed in PSUM f32, so wv carries no
    fp8 quantization error); 8 matmuls per chunk accumulate a [2, 512]
    PSUM tile (row h = scores of half h).  A small DVE cast stages the
    [2,512] rows to bf16, a scatter-DMA (DRAM bounce) drops them into
    a per-batch [128, 64] column tile, and one ACT exp per half batch
    (scale=1/16) produces e in bf16.  Scores are O(1) so no max
    subtraction is needed; bv cancels in softmax and is dropped.
  - Pass B: per 128 timesteps, acc += e_col * encN via the fused
    scalar_tensor_tensor op; even 1024-groups run on the Vector engine,
    odd groups on the (otherwise idle) GpSimd/Pool engine, each with its
    own accumulator.  The ones column of encN makes the same op
    accumulate Z = sum(e).  Finally ctx/Z via a ones^T @ acc matmul,
    reciprocal, and scale.  The very last half-batch accumulates on the
    (otherwise idle) PE instead, shrinking the end-of-kernel tail.
  Pass B lags pass A by half a batch, so the end-of-kernel exposed tail
  is only half a batch of accumulation work.
"""

import os
import sys

for _p in ("/opt/trn_rl_repo", "/root/.axon_site", "/root/.axon_site/_ro/pypackages"):
    if os.path.isdir(_p) and _p not in sys.path:
        sys.path.append(_p)

import numpy as np
import ml_dtypes

import concourse.bass as bass
import concourse.tile as tile
from concourse import bacc, bass_isa, mybir
from concourse.bass_utils import run_bass_kernel_spmd

BF16 = ml_dtypes.bfloat16
FP8 = ml_dtypes.float8_e4m3

B, T, D_ENC, D_HID, D_ATT = 32, 8192, 256, 512, 512
N_CORES = 8
BPC = B // N_CORES          # batches per core = 4
P = 128                     # partitions
TC = 1024                   # pass-A chunk (timesteps)
HTC = TC // 2               # matmul moving-dim half = 512
NCH = T // TC               # pass-A chunks per batch = 8
HCH = NCH // 2              # chunks per half-batch = 4
NU = TC // P                # 128-blocks per chunk = 8
NCOL = T // P               # score columns per batch = 64
KD = D_ENC // P             # k-subtiles of the contraction dim = 2
NJ = D_ATT // P             # a-tiles = 4
DE1 = D_ENC + 1             # encN row with ones column = 257
WSCALE = 16.0               # host-side w1/wv scale (fp8 subnormal dodge)

_PROGRAM_CACHE = {}


def _build_program():
    """Build and finalize the SPMD program (identical on all 8 cores)."""
    if "nc" in _PROGRAM_CACHE:
        return _PROGRAM_CACHE["nc"]

    f32 = mybir.dt.float32
    bf16 = mybir.dt.bfloat16
    fp8 = mybir.dt.float8e4
    Act = mybir.ActivationFunctionType
    Alu = mybir.AluOpType
    DR = mybir.MatmulPerfMode.DoubleRow

    nc = bacc.Bacc("TRN2", target_bir_lowering=False, debug=False,
                   num_devices=N_CORES)

    encT = nc.dram_tensor("encT", [BPC, D_ENC, T], bf16, kind="ExternalInput")
    encN = nc.dram_tensor("encN", [BPC, T, DE1], bf16, kind="ExternalInput")
    w1t = nc.dram_tensor("w1t", [P, KD, NJ, P], bf16, kind="ExternalInput")
    # selector stationaries: [p, jp, h, s(k-pair), m(48: hi at col h,
    # lo at col 32+h)] -- 48 pad keeps the DR pair stride 16B-aligned.
    wvt = nc.dram_tensor("wvt", [P, 2, 2, KD, 48], fp8, kind="ExternalInput")
    vbt = nc.dram_tensor("vbt", [P, BPC * NJ], f32, kind="ExternalInput")
    outd = nc.dram_tensor("out", [BPC, D_ENC], f32, kind="ExternalOutput")
    sscr = nc.dram_tensor("sscr", [BPC * NCH, 2, HTC], bf16)

    with tile.TileContext(nc) as tc:
        import contextlib
        with contextlib.ExitStack() as ctx:
            const = ctx.enter_context(tc.tile_pool(name="const", bufs=1))
            encT_pool = ctx.enter_context(tc.tile_pool(name="encT", bufs=6))
            encN_pool = ctx.enter_context(tc.tile_pool(name="encN", bufs=4))
            tanh_pool = ctx.enter_context(tc.tile_pool(name="tanh", bufs=6))
            ssb_pool = ctx.enter_context(tc.tile_pool(name="ssb", bufs=4))
            stg_pool = ctx.enter_context(tc.tile_pool(name="stg", bufs=2))
            e_pool = ctx.enter_context(tc.tile_pool(name="e", bufs=2))
            sm_pool = ctx.enter_context(tc.tile_pool(name="sm", bufs=4))
            osb_pool = ctx.enter_context(tc.tile_pool(name="osb", bufs=2))
            accv_pool = ctx.enter_context(tc.tile_pool(name="accv", bufs=2))
            accg_pool = ctx.enter_context(tc.tile_pool(name="accg", bufs=2))
            hid_psum = ctx.enter_context(
                tc.tile_pool(name="hid", bufs=3, space="PSUM"))
            sc_psum = ctx.enter_context(
                tc.tile_pool(name="sc", bufs=1, space="PSUM"))
            cf_psum = ctx.enter_context(
                tc.tile_pool(name="cfin", bufs=1, space="PSUM"))

            # constants
            w1_sb = const.tile([P, KD, NJ, P], bf16)
            nc.scalar.dma_start(w1_sb[:], w1t[:])
            wvt_sb = const.tile([P, 2, 2, KD, 48], fp8)
            nc.scalar.dma_start(wvt_sb[:], wvt[:])
            vbt_sb = const.tile([P, BPC * NJ], f32)
            nc.scalar.dma_start(vbt_sb[:], vbt[:])
            ones128 = const.tile([P, 1], f32)
            nc.gpsimd.memset(ones128[:], 1.0)

            stage = {}   # per-batch [128, 64] bf16: scattered 16*score cols
            e_sb = {}    # per-batch [128, 64] bf16: exp(scores)
            acc_v = {}   # per-batch [128, 257] f32: DVE accumulator (even g)
            acc_g = {}   # per-batch [128, 257] f32: DVE accumulator (odd g)
            cfp = {}     # last batch: PE-accumulated [1, 257] psum
            tanh_of = {}  # chunk -> (pair tile jp=0, pair tile jp=1)

            def emit_A_main(b, i):
                encT_t = encT_pool.tile([P, KD, TC], bf16)
                src_ap = (encT[b, :, i * TC:(i + 1) * TC]
                          .rearrange("(k p) t -> p k t", p=P))
                if b == 0 and i < 2:
                    # cold-start: split the first chunks across two queues
                    nc.sync.dma_start(encT_t[:, 0, :], src_ap[:, 0, :])
                    nc.gpsimd.dma_start(encT_t[:, 1, :], src_ap[:, 1, :])
                else:
                    nc.sync.dma_start(encT_t[:], src_ap)
                pair_tiles = []
                for jp in range(NJ // 2):
                    tp = tanh_pool.tile([P, 2, TC], fp8, tag="tanh")
                    pair_tiles.append(tp)
                for j in range(NJ):
                    h_ps = hid_psum.tile([P, TC], f32, tag="hid")
                    for k in range(KD):
                        for h in range(2):
                            nc.tensor.matmul(
                                h_ps[:, h * HTC:(h + 1) * HTC],
                                w1_sb[:, k, j, :],
                                encT_t[:, k, h * HTC:(h + 1) * HTC],
                                start=(k == 0), stop=(k == KD - 1))
                    nc.scalar.activation(
                        pair_tiles[j // 2][:, j % 2, :], h_ps[:], Act.Tanh,
                        bias=vbt_sb[:, b * NJ + j: b * NJ + j + 1])
                tanh_of[i] = pair_tiles

            def emit_A_scores(b, i):
                pair_tiles = tanh_of.pop(i)
                s_ps = sc_psum.tile([34, HTC], f32, tag="sc")
                first, last = (0, 0), (1, 1)
                for h in range(2):
                    for jp in range(NJ // 2):
                        nc.tensor.matmul(
                            s_ps[:],
                            wvt_sb[:, jp, h, :, 0:34],
                            pair_tiles[jp][:, :, h * HTC:(h + 1) * HTC],
                            start=((h, jp) == first),
                            stop=((h, jp) == last),
                            perf_mode=DR)
                # psum rows 0:2 = 16*s_hi (h0,h1), rows 32:34 = 16*s_lo;
                # engines may read only one PSUM operand per op, so stage
                # hi to SBUF first (ACT/DVE alternating), then add lo.
                # Scatter drops the bf16 rows into column form:
                # col m = i*8 + h*4 + u holds scores for t = m*128 + p.
                s_hi = ssb_pool.tile([2, HTC], bf16, tag="shi")
                if i % 2 == 0 or b == BPC - 1:
                    nc.scalar.activation(s_hi[:], s_ps[0:2, :], Act.Copy)
                else:
                    nc.vector.tensor_copy(s_hi[:], s_ps[0:2, :])
                s_sb = ssb_pool.tile([2, HTC], bf16, tag="ssb")
                nc.vector.tensor_add(s_sb[:], s_ps[32:34, :], s_hi[:])
                row = sscr[b * NCH + i]
                nc.sync.dma_start(row, s_sb[:])
                nc.sync.dma_start(
                    stage[b][:, i * NU:(i + 1) * NU],
                    row.rearrange("h (u p) -> p (h u)", p=P))
                if b == BPC - 1 and i >= HCH:
                    # last batch tail: per-chunk exp so the PE tail can
                    # start as soon as each chunk's columns land.
                    nc.scalar.activation(
                        e_sb[b][:, i * NU:(i + 1) * NU],
                        stage[b][:, i * NU:(i + 1) * NU],
                        Act.Exp, scale=1.0 / WSCALE)

            def emit_exp_half(b, half):
                nc.scalar.activation(
                    e_sb[b][:, half * 32:(half + 1) * 32],
                    stage[b][:, half * 32:(half + 1) * 32],
                    Act.Exp, scale=1.0 / WSCALE)

            def emit_acc_init(b):
                acc_v[b] = accv_pool.tile([P, DE1], f32, tag="accv",
                                          name=f"acc_v{b}")
                nc.vector.memset(acc_v[b][:], 0.0)
                acc_g[b] = accg_pool.tile([P, DE1], f32, tag="accg",
                                          name=f"acc_g{b}")
                nc.gpsimd.memset(acc_g[b][:], 0.0)


            def emit_B_group(b, g):
                """One pass-B group = super-chunk g (1024 timesteps)."""
                encN_t = encN_pool.tile([P, NU, DE1], bf16)
                nc.sync.dma_start(
                    encN_t[:],
                    encN[b, g * TC:(g + 1) * TC, :]
                        .rearrange("(n p) d -> p n d", p=P))
                if b == BPC - 1 and g >= 2:
                    # PE is otherwise idle in the kernel tail: accumulate
                    # these groups directly in PSUM via matmuls.
                    if g == 2:
                        cfp["t"] = cf_psum.tile([1, DE1], f32, tag="cfin",
                                                name="cfp_last")
                    for n in range(NU):
                        m = NU * g + n
                        nc.tensor.matmul(
                            cfp["t"][:],
                            e_sb[b][:, m:m + 1],
                            encN_t[:, n, :],
                            start=(g == 2 and n == 0), stop=False)
                    return
                acc = acc_v if g % 2 == 0 else acc_g
                for n in range(NU):
                    m = NU * g + n
                    nc.vector.scalar_tensor_tensor(
                        acc[b][:], encN_t[:, n, :],
                        e_sb[b][:, m:m + 1],
                        acc[b][:],
                        op0=Alu.mult, op1=Alu.add)

            def emit_B_finalize(b):
                if b == BPC - 1:
                    cf = cfp["t"]
                    nc.tensor.matmul(cf[:], ones128[:], acc_v[b][:],
                                     start=False, stop=False)
                    nc.tensor.matmul(cf[:], ones128[:], acc_g[b][:],
                                     start=False, stop=True)
                else:
                    cf = cf_psum.tile([1, DE1], f32, tag="cfin")
                    nc.tensor.matmul(cf[:], ones128[:], acc_v[b][:],
                                     start=True, stop=False)
                    nc.tensor.matmul(cf[:], ones128[:], acc_g[b][:],
                                     start=False, stop=True)
                rzb = sm_pool.tile([1, 1], f32, tag="rz", name=f"rz{b}")
                nc.vector.reciprocal(rzb[:], cf[:, D_ENC:D_ENC + 1])
                o_sb = osb_pool.tile([1, D_ENC], f32, tag="osb")
                nc.vector.tensor_scalar_mul(o_sb[:], cf[:, 0:D_ENC], rzb[:])
                nc.sync.dma_start(outd[b:b + 1, :], o_sb[:])

            for step in range(BPC + 1):
                if step < BPC:
                    stage[step] = stg_pool.tile([P, NCOL], bf16, tag="stg",
                                                name=f"stage{step}")
                    e_sb[step] = e_pool.tile([P, NCOL], bf16, tag="e",
                                             name=f"e_sb{step}")
                for i in range(NCH):
                    if step < BPC:
                        if i > 0:
                            emit_A_scores(step, i - 1)
                        emit_A_main(step, i)
                        if i == HCH:
                            emit_acc_init(step)
                            emit_exp_half(step, 0)
                    if i < HCH:
                        if step >= 1:
                            emit_B_group(step - 1, HCH + i)
                            if i == HCH - 1:
                                emit_B_finalize(step - 1)
                    else:
                        if step < BPC:
                            emit_B_group(step, i - HCH)
                if step < BPC:
                    emit_A_scores(step, NCH - 1)
                    if step != BPC - 1:
                        emit_exp_half(step, 1)

    nc.finalize()
    _PROGRAM_CACHE["nc"] = nc
    return nc


def _prep_inputs(encoder_out, hidden_state_h, hidden_state_c,
                 w1, b1, w2, b2, w3, b3, wv, bv):
    """Host-side sharding + layout prep. Returns per-core input maps."""
    enc = np.asarray(encoder_out, dtype=np.float32)
    # per-batch bias vector: b1 + h@w2 + b2 + c@w3 + b3  (tiny, exact f32)
    vb = (np.asarray(b1, np.float32)
          + np.asarray(hidden_state_h, np.float32) @ np.asarray(w2, np.float32)
          + np.asarray(b2, np.float32)
          + np.asarray(hidden_state_c, np.float32) @ np.asarray(w3, np.float32)
          + np.asarray(b3, np.float32))                        # [B, D_ATT]
    # bv shifts every score equally -> cancels in softmax; dropped.

    w1_h = np.ascontiguousarray(
        np.asarray(w1, np.float32).reshape(KD, P, NJ, P).transpose(1, 0, 2, 3)
    ).astype(BF16)                                             # [128,2,4,128]

    # 16*wv split into fp8 value + fp8 residual; selector layout
    # [p, jp, h, s, m]: col h holds hi of wv[(2*jp+s)*128+p], col 32+h
    # holds the lo residual (summed post-PSUM by the cast's tensor_add).
    wv_f = np.asarray(wv, np.float32).reshape(-1) * WSCALE     # [512]
    wv_hi = wv_f.astype(FP8)
    wv_lo = (wv_f - wv_hi.astype(np.float32)).astype(FP8)
    wvsel = np.zeros((P, 2, 2, KD, 48), np.float32)
    for jp in range(2):
        for s in range(KD):
            a0 = (2 * jp + s) * P
            for h in range(2):
                wvsel[:, jp, h, s, h] = wv_hi[a0:a0 + P].astype(np.float32)
                wvsel[:, jp, h, s, 32 + h] = wv_lo[a0:a0 + P].astype(np.float32)
    wvsel = wvsel.astype(FP8)

    in_maps = []
    for c in range(N_CORES):
        sl = slice(c * BPC, (c + 1) * BPC)
        enc_c = enc[sl]                                        # [4, T, 256]
        encT_c = np.ascontiguousarray(enc_c.transpose(0, 2, 1)).astype(BF16)
        encN_c = np.ascontiguousarray(np.concatenate(
            [enc_c, np.ones((BPC, T, 1), np.float32)], axis=2)).astype(BF16)
        vbt_c = np.ascontiguousarray(
            vb[sl].reshape(BPC, NJ, P).transpose(2, 0, 1).reshape(P, BPC * NJ)
        ).astype(np.float32)
        in_maps.append({
            "encT": encT_c,
            "encN": encN_c,
            "w1t": w1_h,
            "wvt": wvsel,
            "vbt": vbt_c,
        })
    return in_maps


def kernel(**inputs):
    nc = _build_program()
    in_maps = _prep_inputs(**inputs)
    res = run_bass_kernel_spmd(nc, in_maps, list(range(N_CORES)))
    out = np.concatenate([res.results[c]["out"] for c in range(N_CORES)],
                         axis=0)
    return out.astype(np.float32)


if __name__ == "__main__":
    rng = np.random.default_rng(0)
    ins = {
        "encoder_out": rng.standard_normal((B, T, D_ENC), dtype=np.float32),
        "hidden_state_h": rng.standard_normal((B, D_HID), dtype=np.float32),
        "hidden_state_c": rng.standard_normal((B, D_HID), dtype=np.float32),
        "w1": (rng.standard_normal((D_ENC, D_ATT), dtype=np.float32)
               / np.sqrt(D_ENC)),
        "b1": np.zeros(D_ATT, np.float32),
        "w2": (rng.standard_normal((D_HID, D_ATT), dtype=np.float32)
               / np.sqrt(D_HID)),
        "b2": np.zeros(D_ATT, np.float32),
        "w3": (rng.standard_normal((D_HID, D_ATT), dtype=np.float32)
               / np.sqrt(D_HID)),
        "wv": (rng.standard_normal((D_ATT, 1), dtype=np.float32)
               / np.sqrt(D_ATT)),
        "b3": np.zeros(D_ATT, np.float32),
        "bv": np.zeros(1, np.float32),
    }
    got = kernel(**ins)
    print("kernel output:", got.shape, got.dtype)
